# revision 1
# baseline (speedup 1.0000x reference)
"""AngleLoss distributed Trainium2 kernel.

mean(arccos(dot(o,t) / (|o||t|))) over 2,097,152 rows of 3-vectors,
data-parallel over 8 NeuronCores.

Math (per row, division/sign-free):
    prod = (sum o^2) * (sum t^2)
    c    = dot * absrsqrt(prod)            # = cos(theta)
    num  = relu(1 - c)                     # clamped 1-c
    r2   = absrsqrt(|1 - c^2|)
    g    = num * r2                        # = sqrt((1-c)/(1+c)) = tan(theta/2)
    theta = 2 * arctan(g)                  # arctan table covers [0, inf)
Per-core output: [128,1] f32 partial sums of arctan(g); host computes
mean = 2 * total / N.
"""

import sys
import numpy as np

if "/opt/trn_rl_repo" not in sys.path:
    sys.path.insert(0, "/opt/trn_rl_repo")

N_CORES = 8
R_TOTAL = 256 * 8192  # 2097152 rows
PER_CORE = R_TOTAL // N_CORES  # 262144
P = 128
FREE = PER_CORE // P  # 2048

# Tunables
N_TILES = 4
# planes whose square runs on VectorE (f32 tensor_tensor) instead of ScalarE
SQ_ON_VE = ()

_BUILD_CACHE = {}


def _build_nc():
    key = (N_TILES, tuple(SQ_ON_VE))
    if key in _BUILD_CACHE:
        return _BUILD_CACHE[key]

    from contextlib import ExitStack

    import concourse.bass as bass
    import concourse.tile as tile
    from concourse import bacc, mybir

    AF = mybir.ActivationFunctionType
    OP = mybir.AluOpType
    f32 = mybir.dt.float32
    bf16 = mybir.dt.bfloat16

    Ft = FREE // N_TILES

    nc = bacc.Bacc(
        "TRN2", target_bir_lowering=False, debug=False, num_devices=N_CORES
    )
    x = nc.dram_tensor("x", [6 * P, FREE], f32, kind="ExternalInput")
    out = nc.dram_tensor("out", [P, 1], f32, kind="ExternalOutput")

    with tile.TileContext(nc) as tc, ExitStack() as ctx:
        inp = ctx.enter_context(tc.tile_pool(name="inp", bufs=2))
        mid = ctx.enter_context(tc.tile_pool(name="mid", bufs=2))
        per = ctx.enter_context(tc.tile_pool(name="persist", bufs=1))

        g_all = per.tile([P, FREE], bf16)
        t_scr = per.tile([P, FREE], bf16)
        asum = per.tile([P, 1], f32)

        xa = x.ap()

        for i in range(N_TILES):
            sl = bass.ts(i, Ft)

            planes = []
            for j in range(6):
                tj = inp.tile([P, Ft], f32, tag=f"in{j}")
                nc.sync.dma_start(tj[:], xa[j * P : (j + 1) * P, sl])
                planes.append(tj)
            ox, oy, oz, tx, ty, tz = planes

            def tt(name, a, b, op, dtype=bf16):
                t = mid.tile([P, Ft], dtype, tag=name)
                nc.vector.tensor_tensor(t[:], a[:], b[:], op)
                return t

            # dot = ox*tx + oy*ty + oz*tz  (mults f32->bf16, adds bf16)
            mx = tt("mx", ox, tx, OP.mult)
            my = tt("my", oy, ty, OP.mult)
            mz = tt("mz", oz, tz, OP.mult)
            dxy = tt("dxy", mx, my, OP.add)
            dot = tt("dot", dxy, mz, OP.add)

            # squares -> oo, tt
            sq = {}
            for name, pl in (
                ("xo", ox), ("yo", oy), ("zo", oz),
                ("xt", tx), ("yt", ty), ("zt", tz),
            ):
                s = mid.tile([P, Ft], bf16, tag="sq" + name)
                if name in SQ_ON_VE:
                    nc.vector.tensor_tensor(s[:], pl[:], pl[:], OP.mult)
                else:
                    nc.scalar.square(s[:], pl[:])
                sq[name] = s
            oo1 = tt("oo1", sq["xo"], sq["yo"], OP.add)
            oo = tt("oo", oo1, sq["zo"], OP.add)
            tt1 = tt("tt1", sq["xt"], sq["yt"], OP.add)
            ttn = tt("ttn", tt1, sq["zt"], OP.add)

            prod = tt("prod", oo, ttn, OP.mult)
            r1 = mid.tile([P, Ft], bf16, tag="r1")
            nc.scalar.activation(r1[:], prod[:], AF.Abs_reciprocal_sqrt)
            c = tt("c", dot, r1, OP.mult)
            c2 = tt("c2", c, c, OP.mult)
            nump = mid.tile([P, Ft], bf16, tag="nump")
            nc.scalar.activation(nump[:], c[:], AF.Relu, bias=1.0, scale=-1.0)
            r2 = mid.tile([P, Ft], bf16, tag="r2")
            nc.scalar.activation(
                r2[:], c2[:], AF.Abs_reciprocal_sqrt, bias=1.0, scale=-1.0
            )
            nc.vector.tensor_tensor(g_all[:, sl], nump[:], r2[:], OP.mult)

        # one arctan pass over the whole shard; accum_out = per-partition sum
        nc.scalar.activation(
            t_scr[:], g_all[:], AF.Arctan, accum_out=asum[:]
        )
        nc.sync.dma_start(out.ap()[:, :], asum[:])

    nc.compile()
    _BUILD_CACHE[key] = nc
    return nc


def _shard_inputs(outputs, targets):
    o = np.ascontiguousarray(np.asarray(outputs), dtype=np.float32).reshape(-1, 3)
    t = np.ascontiguousarray(np.asarray(targets), dtype=np.float32).reshape(-1, 3)
    in_maps = []
    for cidx in range(N_CORES):
        lo, hi = cidx * PER_CORE, (cidx + 1) * PER_CORE
        oc = o[lo:hi]
        tc_ = t[lo:hi]
        planes = np.empty((6, P, FREE), dtype=np.float32)
        for k in range(3):
            planes[k] = oc[:, k].reshape(P, FREE)
            planes[3 + k] = tc_[:, k].reshape(P, FREE)
        in_maps.append({"x": planes.reshape(6 * P, FREE)})
    return in_maps


LAST_RESULT = None


def kernel(outputs, targets):
    global LAST_RESULT
    import os

    from concourse.bass_utils import run_bass_kernel_spmd

    nc = _build_nc()
    in_maps = _shard_inputs(outputs, targets)
    trace = bool(os.environ.get("ANGLE_KERNEL_TRACE"))
    res = run_bass_kernel_spmd(
        nc, in_maps, core_ids=list(range(N_CORES)), trace=trace
    )
    LAST_RESULT = res
    total = 0.0
    for rmap in res.results:
        total += np.asarray(rmap["out"], dtype=np.float64).sum()
    mean = 2.0 * total / R_TOTAL
    return np.float32(mean)
